# revision 42
# baseline (speedup 1.0000x reference)
"""TRN2 Bass kernel for a fused multi-head attention block (B=2, N=2048,
C=1024, 16 heads, head_dim 64, per-head q/k LayerNorm, out projection).

Sharding: 8 NeuronCores = 2 (batch) x 4 (head groups of 4 heads).
Each core computes qkv for its 4 heads, per-head LN + attention, and a
partial output projection; the host sums the 4 partials per batch
(tensor-parallel unshard) and adds proj bias.

v3 dataflow (all matmul operands bf16, fp32 PSUM accumulation):
  x (host-cast bf16) --DMA-xbar-transpose--> x^T in SBUF (no PE transposes)
  qkv = x^T.T @ W (PE); the per-head LN column sums ride the same matmul
  as 8 extra W columns (host-appended col-sums), so the DVE only computes
  sum-of-squares.  LN stats are batched per 4-chunk group; LN apply (DVE,
  bf16 out) writes a 4-chunk tile that one DMA-xbar transpose moves to
  head-major q^T/k^T (alternating sync/scalar HWDGE queues).
  Attention: per head pair, the two heads' S^T matmuls are row-packed on
  the PE (K=64 at partitions 0-63/64-127) into separate PSUM banks; exp
  per head (ACT, bf16 out); O^T (+ones-row rowsum) accumulates per head.
  O matmuls are software-pipelined one chunk behind the S/exp stream.
  Normalization is deferred off the critical path: unnormalized O^T and
  the rowsum row are copied out (freeing PSUM), then one batched
  reciprocal per slab + gpsimd partition broadcast + DVE multiply feed
  the partial projection (PE, 2 weight loads per token chunk).
"""

import sys

sys.path.insert(0, "/opt/trn_rl_repo")

import numpy as np
import ml_dtypes

# problem shapes (hardcoded; harness contract)
B, NTOK, C = 2, 2048, 1024
NHEADS, HD = 16, 64
EPS = 1e-6
P = 128
KC = C // P  # 8 k-chunks of the C contraction
TCH = NTOK // P  # 16 token chunks
G = NHEADS // 4  # 4 heads per core
GC = G * HD  # 256 cols per section per core
WSEC = 3 * GC + 8  # qkv weight cols + 8 LN col-sum cols
TQ = 512  # query slab width
NSLAB = NTOK // TQ
SCL = HD**-0.5

PROFILE = False  # set True by test harness to capture NTFF exec time
LAST_RESULTS = None

_CACHE = {}


def _build_nc(has_qkv_bias: bool, ln_affine: bool):
    from contextlib import ExitStack
    from concourse import bacc
    import concourse.tile as tile
    from concourse import mybir
    from concourse.bass import ts

    F32 = mybir.dt.float32
    BF16 = mybir.dt.bfloat16
    AX = mybir.AxisListType
    ALU = mybir.AluOpType
    ACTF = mybir.ActivationFunctionType

    from concourse import library_config

    nc = bacc.Bacc("TRN2", target_bir_lowering=False, debug=False)
    x_d = nc.dram_tensor("x_shard", [NTOK, C], BF16, kind="ExternalInput")
    # weights host-preshuffled to [partition, chunk*cols] for linear DMA
    wq_d = nc.dram_tensor("wq_shard", [P, KC * WSEC], BF16, kind="ExternalInput")
    wp_d = nc.dram_tensor("wp_shard", [P, 2 * C], BF16, kind="ExternalInput")
    if has_qkv_bias:
        qb_d = nc.dram_tensor("qb_shard", [1, WSEC], F32, kind="ExternalInput")
    if ln_affine:
        # rows: [qscale rep4 | kscale rep4], [qbias rep4 | kbias rep4]
        ln_d = nc.dram_tensor("ln_rows", [2, 2 * GC], F32, kind="ExternalInput")
    out_d = nc.dram_tensor("out_part", [NTOK, C], F32, kind="ExternalOutput")

    with tile.TileContext(nc) as tc:
        with ExitStack() as ctx:
            persist = ctx.enter_context(tc.tile_pool(name="persist", bufs=1))
            # x^T, slab-major so each DMA-transpose dest is contiguous
            xT = persist.tile([P, NSLAB, KC, TQ], BF16, name="xT")
            # q/k head-major: [p, chunk, sec, tok] sec: 0=q01 1=q23 2=k01 3=k23
            qkT = persist.tile([P, TCH, 4, P], BF16, name="qkT")
            # LN working set (token-major qk PSUM evac) + stats
            qkA = persist.tile([P, TCH, 2 * GC], F32, name="qkA")
            sums = persist.tile([P, TCH, 8], F32, name="sums")
            sumsq = persist.tile([P, TCH, 8], F32, name="sumsq")
            rstd = persist.tile([P, TCH, 8], F32, name="rstd")
            nmr = persist.tile([P, TCH, 8], F32, name="nmr")
            vS = persist.tile([P, TCH, G, HD + 1], BF16, name="vS")
            oT = persist.tile([P, 2, NTOK], BF16, name="oT")
            w_r = persist.tile([P, KC, WSEC], BF16, name="w_r")
            wp_r = persist.tile([P, 2, C], BF16, name="wp_r")
            dml = persist.tile([1, 1], F32, name="dml")

            # ---- stage 1: x^T via DMA xbar transpose; weight loads ride
            # the SWDGE (gpsimd) queue so the sync queue starts on x at once.
            # Slab 0 is split per chunk so the first qkv matmul starts early.
            nc.sync.dma_start(w_r[:], wq_d.rearrange("p (k w) -> p k w", w=WSEC))
            for s in range(NSLAB):
                nc.sync.dma_start_transpose(xT[:, s], x_d[ts(s, TQ), :])
            nc.sync.dma_start(wp_r[:], wp_d.rearrange("p (k w) -> p k w", w=C))

            nc.gpsimd.load_library(library_config.attn)

            with tc.tile_pool(name="init", bufs=1) as initp:
                nc.vector.memset(dml[:], 1.0)
                t_ones = initp.tile([P, TCH, G], F32, name="t_ones")
                nc.vector.memset(t_ones[:], 1.0)
                nc.vector.tensor_copy(vS[:, :, :, HD], t_ones[:])
                if has_qkv_bias:
                    qb1 = initp.tile([1, WSEC], F32, name="qb1")
                    nc.gpsimd.dma_start(qb1[:], qb_d[:])
                    brep = persist.tile([P, WSEC], F32, name="brep")
                    nc.gpsimd.partition_broadcast(brep[:], qb1[:])
                if ln_affine:
                    ln1 = initp.tile([2, 2 * GC], F32, name="ln1")
                    nc.gpsimd.dma_start(ln1[:], ln_d[:])
                    srep = persist.tile([P, 2 * GC], F32, name="srep")
                    lbrep = persist.tile([P, 2 * GC], F32, name="lbrep")
                    nc.gpsimd.partition_broadcast(srep[:], ln1[0:1, :])
                    nc.gpsimd.partition_broadcast(lbrep[:], ln1[1:2, :])

            # ---- stage 2: qkv matmuls, LN (grouped stats), q/k transposes ----
            with (
                tc.tile_pool(name="s2", bufs=3) as sp2,
                tc.tile_pool(name="s2t", bufs=2) as sp2t,
                tc.tile_pool(name="qkvps", bufs=2, space="PSUM") as psQK,
            ):
                def emit_ln(g, ln_pending):
                    for j0, w in ln_pending:
                        gs = slice(4 * g + j0, 4 * g + j0 + w)
                        mean = sp2.tile([P, w, 8], F32, tag="mean")
                        nc.vector.tensor_scalar_mul(
                            mean[:], sums[:, gs], 1.0 / HD
                        )
                        msq = sp2.tile([P, w, 8], F32, tag="msq")
                        nc.vector.tensor_tensor(msq[:], mean[:], mean[:], ALU.mult)
                        varep = sp2.tile([P, w, 8], F32, tag="varep")
                        nc.vector.scalar_tensor_tensor(
                            varep[:], in0=sumsq[:, gs], scalar=1.0 / HD,
                            in1=msq[:], op0=ALU.mult, op1=ALU.subtract,
                        )
                        nc.vector.tensor_scalar_add(varep[:], varep[:], EPS)
                        rv = sp2.tile([P, w, 8], F32, tag="rv")
                        nc.vector.reciprocal(rv[:], varep[:])
                        nc.scalar.activation(rstd[:, gs], rv[:], ACTF.Sqrt)
                        nc.vector.scalar_tensor_tensor(
                            nmr[:, gs], in0=mean[:], scalar=-1.0,
                            in1=rstd[:, gs], op0=ALU.mult, op1=ALU.mult,
                        )
                        # LN apply (bf16 out), then xbar transpose to
                        # head-major
                        qkl4 = sp2t.tile([P, w, 2 * GC], BF16, tag="qkl4")
                        for j in range(w):
                            t = 4 * g + j0 + j
                            a3 = qkA[:, t].rearrange("p (g d) -> p g d", d=HD)
                            nc.vector.tensor_tensor(
                                a3, a3,
                                rstd[:, t, :, None].to_broadcast([P, 8, HD]),
                                ALU.mult,
                            )
                            q3 = qkl4[:, j].rearrange("p (g d) -> p g d", d=HD)
                            nc.vector.tensor_tensor(
                                q3, a3,
                                nmr[:, t, :, None].to_broadcast([P, 8, HD]),
                                ALU.add,
                            )
                            if ln_affine:
                                nc.vector.tensor_tensor(
                                    qkl4[:, j], qkl4[:, j], srep[:], ALU.mult
                                )
                                nc.vector.tensor_tensor(
                                    qkl4[:, j], qkl4[:, j], lbrep[:], ALU.add
                                )
                        nc.sync.dma_start_transpose(
                            qkT[:, 4 * g + j0 : 4 * g + j0 + w], qkl4[:]
                        )


                for g in range(NSLAB):  # group g == slab g (4 chunks)
                    last = g == NSLAB - 1
                    for j in range(4):
                        t = 4 * g + j
                        ln_pending = [(j, 1)] if last else []
                        psA = psQK.tile([P, 2 * GC], F32, tag="psA")
                        psB = psQK.tile([P, GC + 8], F32, tag="psB")
                        for kc in range(KC):
                            lhsT = xT[:, g, kc, ts(j, P)]
                            nc.tensor.matmul(
                                psA[:],
                                lhsT,
                                w_r[:, kc, 0 : 2 * GC],
                                start=(kc == 0),
                                stop=(kc == KC - 1),
                            )
                            nc.tensor.matmul(
                                psB[:],
                                lhsT,
                                w_r[:, kc, 2 * GC : WSEC],
                                start=(kc == 0),
                                stop=(kc == KC - 1),
                            )
                        if has_qkv_bias:
                            nc.vector.tensor_tensor(
                                psA[:], psA[:], brep[:, 0 : 2 * GC], ALU.add
                            )
                            nc.vector.tensor_tensor(
                                psB[:], psB[:], brep[:, 2 * GC : WSEC], ALU.add
                            )
                        sq = sp2.tile([P, 2 * GC], F32, tag="sq")
                        nc.scalar.square(sq[:], psA[:])
                        nc.scalar.copy(qkA[:, t], psA[:])
                        nc.vector.tensor_reduce(
                            sumsq[:, t],
                            sq[:].rearrange("p (g d) -> p g d", d=HD),
                            axis=AX.X,
                            op=ALU.add,
                        )
                        nc.scalar.copy(
                            vS[:, t, :, 0:HD],
                            psB[:, 0:GC].rearrange("p (g d) -> p g d", d=HD),
                        )
                        nc.scalar.copy(sums[:, t], psB[:, GC : GC + 8])
                        emit_ln(g, ln_pending)

                    emit_ln(g, [] if last else [(0, 4)])

            # Warm the exp activation table now: the dummy has no input
            # deps, so the ACT engine issues its table load into the HWDGE
            # queue ahead of the last group's qkl transpose.
            nc.scalar.activation(dml[:], dml[:], ACTF.Exp)

            # ---- stage 3+4: attention (head-pair row-packed S) + proj ----
            with (
                tc.tile_pool(name="s3e", bufs=4) as ep,
                tc.tile_pool(name="s3o", bufs=4) as op_,
                tc.tile_pool(name="s3r", bufs=2) as rp,
                tc.tile_pool(name="sps", bufs=2, space="PSUM") as sps,
                tc.tile_pool(name="ops", bufs=2, space="PSUM") as ops,
                tc.tile_pool(name="pps", bufs=2, space="PSUM") as pps,
                tc.tile_pool(name="s4", bufs=3) as s4p,
            ):
                def emit_proj(s):
                    # partial projection for slab s's 4 token chunks
                    for j in range(4):
                        t = 4 * s + j
                        pp0 = pps.tile([P, 512], F32, tag="pp")
                        pp1 = pps.tile([P, 512], F32, tag="pp")
                        for kc2 in range(2):
                            nc.tensor.matmul(
                                pp0[:], oT[:, kc2, ts(t, P)], wp_r[:, kc2, 0:512],
                                start=(kc2 == 0), stop=(kc2 == 1),
                            )
                            nc.tensor.matmul(
                                pp1[:], oT[:, kc2, ts(t, P)], wp_r[:, kc2, 512:1024],
                                start=(kc2 == 0), stop=(kc2 == 1),
                            )
                        for n2, pp in ((0, pp0), (1, pp1)):
                            ob = s4p.tile([P, 512], F32, tag="ob")
                            nc.vector.tensor_copy(ob[:], pp[:])
                            nc.sync.dma_start(out_d[ts(t, P), ts(n2, 512)], ob[:])

                for s in range(NSLAB):
                    # rowsum rows at 32-aligned partitions (DVE access rule)
                    rs4 = rp.tile([P, TQ], F32, tag="rs4")
                    oUs = {}
                    if True:
                        for pr in range(2):
                            osA = ops.tile([HD + 1, TQ], F32, tag="osum")
                            osB = ops.tile([HD + 1, TQ], F32, tag="osum")
                            prev = None
                            for dk in range(TCH // 2):
                                # two key chunks per S/exp round: FD-1024
                                # exps amortize the ACT per-instr overhead
                                sptA = sps.tile([P, 2, TQ], F32, tag="spt")
                                sptB = sps.tile([P, 2, TQ], F32, tag="spt")
                                for h in range(2):
                                    tk = 2 * dk + h
                                    nc.tensor.matmul(
                                        sptA[:, h],
                                        qkT[0:HD, tk, 2 + pr, :],
                                        qkT[0:HD, ts(s, 4), pr, :],
                                        start=True, stop=True,
                                        tile_position=(0, 0),
                                    )
                                    nc.tensor.matmul(
                                        sptB[:, h],
                                        qkT[HD:P, tk, 2 + pr, :],
                                        qkT[HD:P, ts(s, 4), pr, :],
                                        start=True, stop=True,
                                        tile_position=(64, 0),
                                    )
                                etA = ep.tile([P, 2, TQ], BF16, tag="et")
                                etB = ep.tile([P, 2, TQ], BF16, tag="et")
                                nc.scalar.activation(
                                    etA[:], sptA[:], ACTF.Exp, scale=SCL
                                )
                                nc.scalar.activation(
                                    etB[:], sptB[:], ACTF.Exp, scale=SCL
                                )
                                # O matmuls pipelined one round behind
                                if prev is not None:
                                    pA, pB, pdk = prev
                                    for h in range(2):
                                        tk = 2 * pdk + h
                                        nc.tensor.matmul(
                                            osA[:], vS[:, tk, 2 * pr, :],
                                            pA[:, h],
                                            start=(tk == 0), stop=False,
                                        )
                                        nc.tensor.matmul(
                                            osB[:], vS[:, tk, 2 * pr + 1, :],
                                            pB[:, h],
                                            start=(tk == 0), stop=False,
                                        )
                                prev = (etA, etB, dk)
                            pA, pB, pdk = prev
                            for h in range(2):
                                tk = 2 * pdk + h
                                nc.tensor.matmul(
                                    osA[:], vS[:, tk, 2 * pr, :], pA[:, h],
                                    start=False, stop=(tk == TCH - 1),
                                )
                                nc.tensor.matmul(
                                    osB[:], vS[:, tk, 2 * pr + 1, :], pB[:, h],
                                    start=False, stop=(tk == TCH - 1),
                                )
                            # evacuate unnormalized O^T + rowsum; frees PSUM
                            for a, osm in ((0, osA), (1, osB)):
                                oU = op_.tile([HD, TQ], BF16, tag="oU")
                                nc.vector.tensor_copy(oU[:], osm[0:HD, :])
                                nc.vector.tensor_copy(
                                    rs4[
                                        32 * (2 * pr + a) : 32 * (2 * pr + a)
                                        + 1,
                                        :,
                                    ],
                                    osm[HD : HD + 1, :],
                                )
                                oUs[(pr, a)] = oU
                            if s == NSLAB - 1:
                                # final slab: normalize per pair so pair 0's
                                # chain hides under pair 1's attention
                                rc2 = rp.tile([P, TQ], F32, tag="rc4")
                                nc.vector.reciprocal(
                                    rc2[0:64, :], rs4[0:64, :]
                                ) if pr == 0 else nc.vector.reciprocal(
                                    rc2[64:128, :], rs4[64:128, :]
                                )
                                rcr = rp.tile([1, 2, TQ], F32, tag="rcr")
                                for a in range(2):
                                    nc.vector.tensor_copy(
                                        rcr[:, a],
                                        rc2[
                                            32 * (2 * pr + a) : 32
                                            * (2 * pr + a)
                                            + 1,
                                            :,
                                        ],
                                    )
                                bcr = rp.tile([HD, 2, TQ], F32, tag="bcr")
                                nc.gpsimd.partition_broadcast(bcr[:], rcr[:])
                                for a in range(2):
                                    nc.vector.tensor_tensor(
                                        oT[ts(a, HD), pr, ts(s, TQ)],
                                        oUs[(pr, a)][:],
                                        bcr[:, a],
                                        ALU.mult,
                                    )
                    if s < NSLAB - 1:
                        # batched softmax denominators for the slab's 4 heads
                        rc4 = rp.tile([P, TQ], F32, tag="rc4")
                        nc.vector.reciprocal(rc4[:], rs4[:])
                        # gather the 4 reciprocal rows on partition 0 and
                        # broadcast all heads' scales in one gpsimd op
                        rcr = rp.tile([1, 4, TQ], F32, tag="rcr")
                        for i in range(4):
                            nc.vector.tensor_copy(
                                rcr[:, i], rc4[32 * i : 32 * i + 1, :]
                            )
                        bcr = rp.tile([HD, 4, TQ], F32, tag="bcr")
                        nc.gpsimd.partition_broadcast(bcr[:], rcr[:])
                        for pr in range(2):
                            for a in range(2):
                                nc.vector.tensor_tensor(
                                    oT[ts(a, HD), pr, ts(s, TQ)],
                                    oUs[(pr, a)][:],
                                    bcr[:, 2 * pr + a],
                                    ALU.mult,
                                )
                    emit_proj(s)

    nc.compile()
    return nc


def _get_nc(has_qkv_bias: bool, ln_affine: bool):
    key = (has_qkv_bias, ln_affine)
    if key not in _CACHE:
        _CACHE[key] = _build_nc(*key)
    return _CACHE[key]


def kernel(**inputs) -> np.ndarray:
    global LAST_RESULTS
    from concourse.bass_utils import run_bass_kernel_spmd

    x = np.asarray(inputs["x"], dtype=np.float32)
    qkv_w = np.asarray(inputs["qkv_w"], dtype=np.float32)
    qkv_b = np.asarray(inputs["qkv_b"], dtype=np.float32)
    qn_scale = np.asarray(inputs["qn_scale"], dtype=np.float32)
    qn_bias = np.asarray(inputs["qn_bias"], dtype=np.float32)
    kn_scale = np.asarray(inputs["kn_scale"], dtype=np.float32)
    kn_bias = np.asarray(inputs["kn_bias"], dtype=np.float32)
    proj_w = np.asarray(inputs["proj_w"], dtype=np.float32)
    proj_b = np.asarray(inputs["proj_b"], dtype=np.float32)

    has_qkv_bias = bool(np.any(qkv_b != 0))
    ln_affine = not (
        np.all(qn_scale == 1)
        and np.all(kn_scale == 1)
        and np.all(qn_bias == 0)
        and np.all(kn_bias == 0)
    )
    nc = _get_nc(has_qkv_bias, ln_affine)

    bf16 = ml_dtypes.bfloat16
    in_maps = []
    for c in range(8):
        b, g = divmod(c, 4)
        cs = slice(g * GC, (g + 1) * GC)
        wqk = np.concatenate([qkv_w[:, cs], qkv_w[:, C:][:, cs]], axis=1).astype(bf16)
        # LN column sums ride the qkv matmul as extra output columns;
        # match the bf16-rounded weights the matmul actually uses
        wsum = wqk.astype(np.float32).reshape(C, 8, HD).sum(-1)
        wq = np.concatenate(
            [
                wqk.astype(np.float32),
                qkv_w[:, 2 * C :][:, cs],
                wsum,
            ],
            axis=1,
        ).astype(bf16)
        # preshuffle to [partition, chunk*cols] for a linear DMA burst
        wq = np.ascontiguousarray(
            wq.reshape(KC, P, WSEC).transpose(1, 0, 2).reshape(P, KC * WSEC)
        )
        wp = np.ascontiguousarray(
            proj_w[cs, :].astype(bf16).reshape(2, P, C).transpose(1, 0, 2)
            .reshape(P, 2 * C)
        )
        m = {
            "x_shard": np.ascontiguousarray(x[b]).astype(bf16),
            "wq_shard": wq,
            "wp_shard": wp,
        }
        if has_qkv_bias:
            qbk = np.concatenate([qkv_b[cs], qkv_b[C:][cs]])
            m["qb_shard"] = np.concatenate(
                [qbk, qkv_b[2 * C :][cs], qbk.reshape(8, HD).sum(-1)]
            ).reshape(1, WSEC).astype(np.float32)
        if ln_affine:
            m["ln_rows"] = np.stack(
                [
                    np.concatenate([np.tile(qn_scale, G), np.tile(kn_scale, G)]),
                    np.concatenate([np.tile(qn_bias, G), np.tile(kn_bias, G)]),
                ]
            ).astype(np.float32)
        in_maps.append(m)

    res = run_bass_kernel_spmd(
        nc, in_maps, core_ids=list(range(8)), trace=PROFILE
    )
    LAST_RESULTS = res

    out = np.empty((B, NTOK, C), dtype=np.float32)
    for b in range(B):
        acc = res.results[4 * b]["out_part"].astype(np.float32).copy()
        for g in range(1, 4):
            acc += res.results[4 * b + g]["out_part"]
        out[b] = acc + proj_b[None, :]
    return out


# revision 43
# speedup vs baseline: 1.2182x; 1.2182x over previous
"""TRN2 Bass kernel for a fused multi-head attention block (B=2, N=2048,
C=1024, 16 heads, head_dim 64, per-head q/k LayerNorm, out projection).

Sharding: 8 NeuronCores = 2 (batch) x 4 (head groups of 4 heads).
Each core computes qkv for its 4 heads, per-head LN + attention, and a
partial output projection; the host sums the 4 partials per batch
(tensor-parallel unshard) and adds proj bias.

v3 dataflow (all matmul operands bf16, fp32 PSUM accumulation):
  x (host-cast bf16) --DMA-xbar-transpose--> x^T in SBUF (no PE transposes)
  qkv = x^T.T @ W (PE); the per-head LN column sums ride the same matmul
  as 8 extra W columns (host-appended col-sums), so the DVE only computes
  sum-of-squares.  LN stats are batched per 4-chunk group; LN apply (DVE,
  bf16 out) writes a 4-chunk tile that one DMA-xbar transpose moves to
  head-major q^T/k^T (alternating sync/scalar HWDGE queues).
  Attention: per head pair, the two heads' S^T matmuls are row-packed on
  the PE (K=64 at partitions 0-63/64-127) into separate PSUM banks; exp
  per head (ACT, bf16 out); O^T (+ones-row rowsum) accumulates per head.
  O matmuls are software-pipelined one chunk behind the S/exp stream.
  Normalization is deferred off the critical path: unnormalized O^T and
  the rowsum row are copied out (freeing PSUM), then one batched
  reciprocal per slab + gpsimd partition broadcast + DVE multiply feed
  the partial projection (PE, 2 weight loads per token chunk).
"""

import sys

sys.path.insert(0, "/opt/trn_rl_repo")

import numpy as np
import ml_dtypes

# problem shapes (hardcoded; harness contract)
B, NTOK, C = 2, 2048, 1024
NHEADS, HD = 16, 64
EPS = 1e-6
P = 128
KC = C // P  # 8 k-chunks of the C contraction
TCH = NTOK // P  # 16 token chunks
G = NHEADS // 4  # 4 heads per core
GC = G * HD  # 256 cols per section per core
WSEC = 3 * GC + 8  # qkv weight cols + 8 LN col-sum cols
TQ = 512  # query slab width
NSLAB = NTOK // TQ
SCL = HD**-0.5

PROFILE = False  # set True by test harness to capture NTFF exec time
LAST_RESULTS = None

_CACHE = {}


def _build_nc(has_qkv_bias: bool, ln_affine: bool):
    from contextlib import ExitStack
    from concourse import bacc
    import concourse.tile as tile
    from concourse import mybir
    from concourse.bass import ts

    F32 = mybir.dt.float32
    BF16 = mybir.dt.bfloat16
    AX = mybir.AxisListType
    ALU = mybir.AluOpType
    ACTF = mybir.ActivationFunctionType

    from concourse import library_config

    nc = bacc.Bacc("TRN2", target_bir_lowering=False, debug=False)
    x_d = nc.dram_tensor("x_shard", [NTOK, C], BF16, kind="ExternalInput")
    # weights host-preshuffled to [partition, chunk*cols] for linear DMA
    wq_d = nc.dram_tensor("wq_shard", [P, KC * WSEC], BF16, kind="ExternalInput")
    wp_d = nc.dram_tensor("wp_shard", [P, 2 * C], BF16, kind="ExternalInput")
    if has_qkv_bias:
        qb_d = nc.dram_tensor("qb_shard", [1, WSEC], F32, kind="ExternalInput")
    if ln_affine:
        # rows: [qscale rep4 | kscale rep4], [qbias rep4 | kbias rep4]
        ln_d = nc.dram_tensor("ln_rows", [2, 2 * GC], F32, kind="ExternalInput")
    out_d = nc.dram_tensor("out_part", [NTOK, C], F32, kind="ExternalOutput")

    with tile.TileContext(nc) as tc:
        with ExitStack() as ctx:
            persist = ctx.enter_context(tc.tile_pool(name="persist", bufs=1))
            # x^T, slab-major so each DMA-transpose dest is contiguous
            xT = persist.tile([P, NSLAB, KC, TQ], BF16, name="xT")
            # q/k head-major: [p, chunk, sec, tok] sec: 0=q01 1=q23 2=k01 3=k23
            qkT = persist.tile([P, TCH, 4, P], BF16, name="qkT")
            # LN working set (token-major qk PSUM evac) + stats
            qkA = persist.tile([P, TCH, 2 * GC], F32, name="qkA")
            sums = persist.tile([P, TCH, 8], F32, name="sums")
            sumsq = persist.tile([P, TCH, 8], F32, name="sumsq")
            rstd = persist.tile([P, TCH, 8], F32, name="rstd")
            nmr = persist.tile([P, TCH, 8], F32, name="nmr")
            vS = persist.tile([P, TCH, G, HD + 1], BF16, name="vS")
            oT = persist.tile([P, 2, NTOK], BF16, name="oT")
            w_r = persist.tile([P, KC, WSEC], BF16, name="w_r")
            wp_r = persist.tile([P, 2, C], BF16, name="wp_r")
            dml = persist.tile([1, 1], F32, name="dml")

            # ---- stage 1: x^T via DMA xbar transpose; weight loads ride
            # the SWDGE (gpsimd) queue so the sync queue starts on x at once.
            # Slab 0 is split per chunk so the first qkv matmul starts early.
            nc.sync.dma_start(w_r[:], wq_d.rearrange("p (k w) -> p k w", w=WSEC))
            for s in range(NSLAB):
                nc.sync.dma_start_transpose(xT[:, s], x_d[ts(s, TQ), :])
            nc.sync.dma_start(wp_r[:], wp_d.rearrange("p (k w) -> p k w", w=C))

            nc.gpsimd.load_library(library_config.attn)

            with tc.tile_pool(name="init", bufs=1) as initp:
                nc.vector.memset(dml[:], 1.0)
                t_ones = initp.tile([P, TCH, G], F32, name="t_ones")
                nc.vector.memset(t_ones[:], 1.0)
                nc.vector.tensor_copy(vS[:, :, :, HD], t_ones[:])
                if has_qkv_bias:
                    qb1 = initp.tile([1, WSEC], F32, name="qb1")
                    nc.gpsimd.dma_start(qb1[:], qb_d[:])
                    brep = persist.tile([P, WSEC], F32, name="brep")
                    nc.gpsimd.partition_broadcast(brep[:], qb1[:])
                if ln_affine:
                    ln1 = initp.tile([2, 2 * GC], F32, name="ln1")
                    nc.gpsimd.dma_start(ln1[:], ln_d[:])
                    srep = persist.tile([P, 2 * GC], F32, name="srep")
                    lbrep = persist.tile([P, 2 * GC], F32, name="lbrep")
                    nc.gpsimd.partition_broadcast(srep[:], ln1[0:1, :])
                    nc.gpsimd.partition_broadcast(lbrep[:], ln1[1:2, :])

            # ---- stage 2: qkv matmuls, LN (grouped stats), q/k transposes ----
            with (
                tc.tile_pool(name="s2", bufs=3) as sp2,
                tc.tile_pool(name="s2t", bufs=2) as sp2t,
                tc.tile_pool(name="qkvps", bufs=2, space="PSUM") as psQK,
            ):
                def emit_ln(g, ln_pending):
                    for j0, w in ln_pending:
                        gs = slice(4 * g + j0, 4 * g + j0 + w)
                        mean = sp2.tile([P, w, 8], F32, tag="mean")
                        nc.vector.tensor_scalar_mul(
                            mean[:], sums[:, gs], 1.0 / HD
                        )
                        msq = sp2.tile([P, w, 8], F32, tag="msq")
                        nc.vector.tensor_tensor(msq[:], mean[:], mean[:], ALU.mult)
                        varep = sp2.tile([P, w, 8], F32, tag="varep")
                        nc.vector.scalar_tensor_tensor(
                            varep[:], in0=sumsq[:, gs], scalar=1.0 / HD,
                            in1=msq[:], op0=ALU.mult, op1=ALU.subtract,
                        )
                        nc.vector.tensor_scalar_add(varep[:], varep[:], EPS)
                        rv = sp2.tile([P, w, 8], F32, tag="rv")
                        nc.vector.reciprocal(rv[:], varep[:])
                        nc.scalar.activation(rstd[:, gs], rv[:], ACTF.Sqrt)
                        nc.vector.scalar_tensor_tensor(
                            nmr[:, gs], in0=mean[:], scalar=-1.0,
                            in1=rstd[:, gs], op0=ALU.mult, op1=ALU.mult,
                        )
                        # LN apply (bf16 out), then xbar transpose to
                        # head-major
                        qkl4 = sp2t.tile([P, w, 2 * GC], BF16, tag="qkl4")
                        for j in range(w):
                            t = 4 * g + j0 + j
                            a3 = qkA[:, t].rearrange("p (g d) -> p g d", d=HD)
                            nc.vector.tensor_tensor(
                                a3, a3,
                                rstd[:, t, :, None].to_broadcast([P, 8, HD]),
                                ALU.mult,
                            )
                            q3 = qkl4[:, j].rearrange("p (g d) -> p g d", d=HD)
                            nc.vector.tensor_tensor(
                                q3, a3,
                                nmr[:, t, :, None].to_broadcast([P, 8, HD]),
                                ALU.add,
                            )
                            if ln_affine:
                                nc.vector.tensor_tensor(
                                    qkl4[:, j], qkl4[:, j], srep[:], ALU.mult
                                )
                                nc.vector.tensor_tensor(
                                    qkl4[:, j], qkl4[:, j], lbrep[:], ALU.add
                                )
                        nc.sync.dma_start_transpose(
                            qkT[:, 4 * g + j0 : 4 * g + j0 + w], qkl4[:]
                        )


                for g in range(NSLAB):  # group g == slab g (4 chunks)
                    last = g == NSLAB - 1
                    for j in range(4):
                        t = 4 * g + j
                        ln_pending = [(j, 1)] if last else []
                        psA = psQK.tile([P, 2 * GC], F32, tag="psA")
                        psB = psQK.tile([P, GC + 8], F32, tag="psB")
                        for kc in range(KC):
                            lhsT = xT[:, g, kc, ts(j, P)]
                            nc.tensor.matmul(
                                psA[:],
                                lhsT,
                                w_r[:, kc, 0 : 2 * GC],
                                start=(kc == 0),
                                stop=(kc == KC - 1),
                            )
                            nc.tensor.matmul(
                                psB[:],
                                lhsT,
                                w_r[:, kc, 2 * GC : WSEC],
                                start=(kc == 0),
                                stop=(kc == KC - 1),
                            )
                        if has_qkv_bias:
                            nc.vector.tensor_tensor(
                                psA[:], psA[:], brep[:, 0 : 2 * GC], ALU.add
                            )
                            nc.vector.tensor_tensor(
                                psB[:], psB[:], brep[:, 2 * GC : WSEC], ALU.add
                            )
                        sq = sp2.tile([P, 2 * GC], F32, tag="sq")
                        nc.scalar.square(sq[:], psA[:])
                        nc.scalar.copy(qkA[:, t], psA[:])
                        nc.vector.tensor_reduce(
                            sumsq[:, t],
                            sq[:].rearrange("p (g d) -> p g d", d=HD),
                            axis=AX.X,
                            op=ALU.add,
                        )
                        nc.scalar.copy(
                            vS[:, t, :, 0:HD],
                            psB[:, 0:GC].rearrange("p (g d) -> p g d", d=HD),
                        )
                        nc.scalar.copy(sums[:, t], psB[:, GC : GC + 8])
                        emit_ln(g, ln_pending)

                    emit_ln(g, [] if last else [(0, 4)])

            # Warm the exp activation table now: the dummy has no input
            # deps, so the ACT engine issues its table load into the HWDGE
            # queue ahead of the last group's qkl transpose.
            nc.scalar.activation(dml[:], dml[:], ACTF.Exp)

            # ---- stage 3+4: attention (head-pair row-packed S) + proj ----
            with (
                tc.tile_pool(name="s3e", bufs=4) as ep,
                tc.tile_pool(name="s3o", bufs=4) as op_,
                tc.tile_pool(name="s3r", bufs=2) as rp,
                tc.tile_pool(name="sps", bufs=2, space="PSUM") as sps,
                tc.tile_pool(name="ops", bufs=2, space="PSUM") as ops,
                tc.tile_pool(name="pps", bufs=2, space="PSUM") as pps,
                tc.tile_pool(name="s4", bufs=3) as s4p,
            ):
                def emit_proj(s):
                    # partial projection for slab s's 4 token chunks
                    for j in range(4):
                        t = 4 * s + j
                        pp0 = pps.tile([P, 512], F32, tag="pp")
                        pp1 = pps.tile([P, 512], F32, tag="pp")
                        for kc2 in range(2):
                            nc.tensor.matmul(
                                pp0[:], oT[:, kc2, ts(t, P)], wp_r[:, kc2, 0:512],
                                start=(kc2 == 0), stop=(kc2 == 1),
                            )
                            nc.tensor.matmul(
                                pp1[:], oT[:, kc2, ts(t, P)], wp_r[:, kc2, 512:1024],
                                start=(kc2 == 0), stop=(kc2 == 1),
                            )
                        for n2, pp in ((0, pp0), (1, pp1)):
                            ob = s4p.tile([P, 512], F32, tag="ob")
                            nc.vector.tensor_copy(ob[:], pp[:])
                            nc.sync.dma_start(out_d[ts(t, P), ts(n2, 512)], ob[:])

                for s in range(NSLAB):
                    # rowsum rows at 32-aligned partitions (DVE access rule)
                    rs4 = rp.tile([P, TQ], F32, tag="rs4")
                    oUs = {}
                    if True:
                        for pr in range(2):
                            osA = ops.tile([HD + 1, TQ], F32, tag="osum")
                            osB = ops.tile([HD + 1, TQ], F32, tag="osum")
                            prev = None
                            for dk in range(TCH // 2):
                                # two key chunks per S/exp round: FD-1024
                                # exps amortize the ACT per-instr overhead
                                sptA = sps.tile([P, 2, TQ], F32, tag="spt")
                                sptB = sps.tile([P, 2, TQ], F32, tag="spt")
                                for h in range(2):
                                    tk = 2 * dk + h
                                    nc.tensor.matmul(
                                        sptA[:, h],
                                        qkT[0:HD, tk, 2 + pr, :],
                                        qkT[0:HD, ts(s, 4), pr, :],
                                        start=True, stop=True,
                                        tile_position=(0, 0),
                                    )
                                    nc.tensor.matmul(
                                        sptB[:, h],
                                        qkT[HD:P, tk, 2 + pr, :],
                                        qkT[HD:P, ts(s, 4), pr, :],
                                        start=True, stop=True,
                                        tile_position=(64, 0),
                                    )
                                etA = ep.tile([P, 2, TQ], BF16, tag="et")
                                etB = ep.tile([P, 2, TQ], BF16, tag="et")
                                nc.scalar.activation(
                                    etA[:], sptA[:], ACTF.Exp, scale=SCL
                                )
                                nc.scalar.activation(
                                    etB[:], sptB[:], ACTF.Exp, scale=SCL
                                )
                                # O matmuls pipelined one round behind
                                if prev is not None:
                                    pA, pB, pdk = prev
                                    for h in range(2):
                                        tk = 2 * pdk + h
                                        nc.tensor.matmul(
                                            osA[:], vS[:, tk, 2 * pr, :],
                                            pA[:, h],
                                            start=(tk == 0), stop=False,
                                        )
                                        nc.tensor.matmul(
                                            osB[:], vS[:, tk, 2 * pr + 1, :],
                                            pB[:, h],
                                            start=(tk == 0), stop=False,
                                        )
                                prev = (etA, etB, dk)
                            pA, pB, pdk = prev
                            for h in range(2):
                                tk = 2 * pdk + h
                                nc.tensor.matmul(
                                    osA[:], vS[:, tk, 2 * pr, :], pA[:, h],
                                    start=False, stop=(tk == TCH - 1),
                                )
                                nc.tensor.matmul(
                                    osB[:], vS[:, tk, 2 * pr + 1, :], pB[:, h],
                                    start=False, stop=(tk == TCH - 1),
                                )
                            # evacuate unnormalized O^T + rowsum; frees PSUM
                            for a, osm in ((0, osA), (1, osB)):
                                oU = op_.tile([HD, TQ], BF16, tag="oU")
                                nc.vector.tensor_copy(oU[:], osm[0:HD, :])
                                nc.vector.tensor_copy(
                                    rs4[
                                        32 * (2 * pr + a) : 32 * (2 * pr + a)
                                        + 1,
                                        :,
                                    ],
                                    osm[HD : HD + 1, :],
                                )
                                oUs[(pr, a)] = oU
                    # batched softmax denominators for the slab's 4 heads
                    rc4 = rp.tile([P, TQ], F32, tag="rc4")
                    nc.vector.reciprocal(rc4[:], rs4[:])
                    # gather the 4 reciprocal rows on partition 0 and
                    # broadcast all heads' scales in one gpsimd op
                    rcr = rp.tile([1, 4, TQ], F32, tag="rcr")
                    for i in range(4):
                        nc.vector.tensor_copy(
                            rcr[:, i], rc4[32 * i : 32 * i + 1, :]
                        )
                    bcr = rp.tile([HD, 4, TQ], F32, tag="bcr")
                    nc.gpsimd.partition_broadcast(bcr[:], rcr[:])
                    for pr in range(2):
                        for a in range(2):
                            nc.vector.tensor_tensor(
                                oT[ts(a, HD), pr, ts(s, TQ)],
                                oUs[(pr, a)][:],
                                bcr[:, 2 * pr + a],
                                ALU.mult,
                            )
                    emit_proj(s)

    nc.compile()
    return nc


def _get_nc(has_qkv_bias: bool, ln_affine: bool):
    key = (has_qkv_bias, ln_affine)
    if key not in _CACHE:
        _CACHE[key] = _build_nc(*key)
    return _CACHE[key]


def kernel(**inputs) -> np.ndarray:
    global LAST_RESULTS
    from concourse.bass_utils import run_bass_kernel_spmd

    x = np.asarray(inputs["x"], dtype=np.float32)
    qkv_w = np.asarray(inputs["qkv_w"], dtype=np.float32)
    qkv_b = np.asarray(inputs["qkv_b"], dtype=np.float32)
    qn_scale = np.asarray(inputs["qn_scale"], dtype=np.float32)
    qn_bias = np.asarray(inputs["qn_bias"], dtype=np.float32)
    kn_scale = np.asarray(inputs["kn_scale"], dtype=np.float32)
    kn_bias = np.asarray(inputs["kn_bias"], dtype=np.float32)
    proj_w = np.asarray(inputs["proj_w"], dtype=np.float32)
    proj_b = np.asarray(inputs["proj_b"], dtype=np.float32)

    has_qkv_bias = bool(np.any(qkv_b != 0))
    ln_affine = not (
        np.all(qn_scale == 1)
        and np.all(kn_scale == 1)
        and np.all(qn_bias == 0)
        and np.all(kn_bias == 0)
    )
    nc = _get_nc(has_qkv_bias, ln_affine)

    bf16 = ml_dtypes.bfloat16
    in_maps = []
    for c in range(8):
        b, g = divmod(c, 4)
        cs = slice(g * GC, (g + 1) * GC)
        wqk = np.concatenate([qkv_w[:, cs], qkv_w[:, C:][:, cs]], axis=1).astype(bf16)
        # LN column sums ride the qkv matmul as extra output columns;
        # match the bf16-rounded weights the matmul actually uses
        wsum = wqk.astype(np.float32).reshape(C, 8, HD).sum(-1)
        wq = np.concatenate(
            [
                wqk.astype(np.float32),
                qkv_w[:, 2 * C :][:, cs],
                wsum,
            ],
            axis=1,
        ).astype(bf16)
        # preshuffle to [partition, chunk*cols] for a linear DMA burst
        wq = np.ascontiguousarray(
            wq.reshape(KC, P, WSEC).transpose(1, 0, 2).reshape(P, KC * WSEC)
        )
        wp = np.ascontiguousarray(
            proj_w[cs, :].astype(bf16).reshape(2, P, C).transpose(1, 0, 2)
            .reshape(P, 2 * C)
        )
        m = {
            "x_shard": np.ascontiguousarray(x[b]).astype(bf16),
            "wq_shard": wq,
            "wp_shard": wp,
        }
        if has_qkv_bias:
            qbk = np.concatenate([qkv_b[cs], qkv_b[C:][cs]])
            m["qb_shard"] = np.concatenate(
                [qbk, qkv_b[2 * C :][cs], qbk.reshape(8, HD).sum(-1)]
            ).reshape(1, WSEC).astype(np.float32)
        if ln_affine:
            m["ln_rows"] = np.stack(
                [
                    np.concatenate([np.tile(qn_scale, G), np.tile(kn_scale, G)]),
                    np.concatenate([np.tile(qn_bias, G), np.tile(kn_bias, G)]),
                ]
            ).astype(np.float32)
        in_maps.append(m)

    res = run_bass_kernel_spmd(
        nc, in_maps, core_ids=list(range(8)), trace=PROFILE
    )
    LAST_RESULTS = res

    out = np.empty((B, NTOK, C), dtype=np.float32)
    for b in range(B):
        acc = res.results[4 * b]["out_part"].astype(np.float32).copy()
        for g in range(1, 4):
            acc += res.results[4 * b + g]["out_part"]
        out[b] = acc + proj_b[None, :]
    return out
